# revision 16
# baseline (speedup 1.0000x reference)
"""Neural CDE Euler scan on 8 Trainium2 NeuronCores.

Data-parallel: batch 512 is split 8 x 64. Each core runs the full
200-step scan on its 64 rows with the MLP weights resident in SBUF.

Per-core layout ("layout B"): activations live feature-on-partition as
[feat, batch] tiles; MLP weights are the stationary matmul operand
(fp32, exact), activations stream as the moving operand. The control
increment dX is precomputed on host (bit-exact replication of the
reference's searchsorted/f32 arithmetic) and shipped per-step as
d-major broadcast rows. The Lorenz-96 drift is computed batch-major
(free-dim rolls) via a pair of PE transposes, off the critical chain.
"""

import os
import sys
import time
import hashlib
import shutil

import numpy as np

B, H, D, W, N = 512, 128, 16, 512, 201
DT = np.float32(0.01)
F_LORENZ = np.float32(8.0)
STEPS = int(os.environ.get("NEURALCDE_STEPS", "200"))
NCORES = 8
BL = B // NCORES  # 64 rows per core

_TRN_PATHS = ["/opt/trn_rl_repo", "/root/.axon_site/_ro/trn_rl_repo"]

_STATE = {}


def _import_concourse():
    for p in _TRN_PATHS:
        if os.path.isdir(p) and p not in sys.path:
            sys.path.insert(0, p)
    import concourse.bass as bass  # noqa
    import concourse.tile as tile  # noqa
    from concourse import bacc, mybir  # noqa
    from concourse.bass_utils import run_bass_kernel_spmd  # noqa

    return bass, tile, bacc, mybir, run_bass_kernel_spmd


def _patch_neff_cache():
    """Content-keyed NEFF cache so fresh processes skip walrus compile."""
    import concourse.bass_utils as bu
    import concourse.bass2jax as b2j

    if getattr(bu, "_neuralcde_cache_patched", False):
        return
    orig = bu.compile_bir_kernel
    cache_dir = os.environ.get("NEURALCDE_NEFF_CACHE", "/tmp/neuralcde_neff_cache")
    os.makedirs(cache_dir, exist_ok=True)

    def cached(bir_json, tmpdir, neff_name="file.neff"):
        data = bir_json if isinstance(bir_json, bytes) else bir_json.encode()
        key = hashlib.sha256(data).hexdigest()
        cp = os.path.join(cache_dir, key + ".neff")
        if os.path.exists(cp):
            dst_dir = os.path.join(tmpdir, "sg00")
            os.makedirs(dst_dir, exist_ok=True)
            dst = os.path.join(dst_dir, neff_name)
            shutil.copyfile(cp, dst)
            return dst
        path = orig(bir_json, tmpdir, neff_name=neff_name)
        try:
            shutil.copyfile(path, cp + f".tmp{os.getpid()}")
            os.replace(cp + f".tmp{os.getpid()}", cp)
        except OSError:
            pass
        return path

    bu.compile_bir_kernel = cached
    if getattr(b2j, "compile_bir_kernel", None) is orig:
        b2j.compile_bir_kernel = cached
    bu._neuralcde_cache_patched = True


def _build(steps):
    """Build + bacc-compile the per-core SPMD program (same on all cores)."""
    bass, tile, bacc, mybir, _ = _import_concourse()
    from contextlib import ExitStack

    f32 = mybir.dt.float32
    AF = mybir.ActivationFunctionType
    ALU = mybir.AluOpType

    nc = bacc.Bacc("TRN2", target_bir_lowering=False, debug=False)

    u0t_d = nc.dram_tensor("u0t", [H, BL], f32, kind="ExternalInput").ap()
    w0s_d = nc.dram_tensor("w0s", [128, 512], f32, kind="ExternalInput").ap()
    w1s_d = nc.dram_tensor("w1s", [128, 2048], f32, kind="ExternalInput").ap()
    w2s_d = nc.dram_tensor("w2s", [128, 8192], f32, kind="ExternalInput").ap()
    dxt_d = nc.dram_tensor("dxt", [steps, 1088], f32, kind="ExternalInput").ap()
    idn_d = nc.dram_tensor("idn", [128, 128], f32, kind="ExternalInput").ap()
    out_d = nc.dram_tensor("out", [H, BL], f32, kind="ExternalOutput").ap()

    with tile.TileContext(nc) as tc, ExitStack() as ctx:
        const = ctx.enter_context(tc.tile_pool(name="const", bufs=1))
        state = ctx.enter_context(tc.tile_pool(name="state", bufs=2))
        work = ctx.enter_context(tc.tile_pool(name="work", bufs=2))
        rowp = ctx.enter_context(tc.tile_pool(name="rowp", bufs=4))
        dxrp = ctx.enter_context(tc.tile_pool(name="dxrp", bufs=3))
        ph1 = ctx.enter_context(tc.tile_pool(name="ph1", bufs=1, space="PSUM"))
        ph2 = ctx.enter_context(tc.tile_pool(name="ph2", bufs=1, space="PSUM"))
        po = ctx.enter_context(tc.tile_pool(name="po", bufs=2, space="PSUM"))
        pt = ctx.enter_context(tc.tile_pool(name="pt", bufs=2, space="PSUM"))

        w0s = const.tile([128, 512], f32)
        w1s = const.tile([128, 2048], f32)
        w2s = const.tile([128, 8192], f32)
        idn = const.tile([128, 128], f32)
        u0t = const.tile([H, BL], f32)
        nc.sync.dma_start(out=w0s[:], in_=w0s_d[:])
        nc.sync.dma_start(out=w1s[:], in_=w1s_d[:])
        nc.sync.dma_start(out=w2s[:], in_=w2s_d[:])
        nc.sync.dma_start(out=idn[:], in_=idn_d[:])
        nc.sync.dma_start(out=u0t[:], in_=u0t_d[:])

        u_t = u0t

        def softplus(dst, psrc, tagbase):
            """dst = relu(psrc) + ln(1 + exp(-|psrc|)), 2 column chunks."""
            ab = work.tile([128, 256], f32, tag=f"{tagbase}_ab")
            ex = work.tile([128, 256], f32, tag=f"{tagbase}_ex")
            ll = work.tile([128, 256], f32, tag=f"{tagbase}_ll")
            for c in range(2):
                cs = slice(c * 128, (c + 1) * 128)
                nc.scalar.activation(ab[:, cs], psrc[:, cs], AF.Abs)
                nc.scalar.activation(ex[:, cs], ab[:, cs], AF.Exp, scale=-1.0)
                nc.scalar.activation(ll[:, cs], ex[:, cs], AF.Ln, bias=1.0)
                # dst = max(psrc, 0) + ll
                nc.vector.scalar_tensor_tensor(
                    dst[:, cs], psrc[:, cs], 0.0, ll[:, cs], ALU.max, ALU.add
                )

        for s in range(steps):
            # ---- dX row for this step: DMA to partition 0, broadcast to 128
            # cols 0:1024 = dX d-major (d*64+b); cols 1024:1088 = sum_d dX[b,d]
            dxrow = rowp.tile([1, 1088], f32, tag="dxrow")
            nc.sync.dma_start(out=dxrow[:], in_=dxt_d[s : s + 1, :])
            dxr = dxrp.tile([128, 1088], f32, tag="dxr")
            nc.gpsimd.partition_broadcast(dxr[:], dxrow[:])

            # ---- mm1: h1T[512,64] = W0 @ uT, 4 M-tiles packed in one bank
            ps1 = ph1.tile([128, 256], f32, tag="ps1")
            for m in range(4):
                nc.tensor.matmul(
                    ps1[:, m * 64 : (m + 1) * 64],
                    lhsT=w0s[:, m * 128 : (m + 1) * 128],
                    rhs=u_t[:],
                    start=(m == 0),
                    stop=(m == 3),
                )

            # ---- lorenz drift (off critical chain): batch-major via transposes
            ptu = pt.tile([BL, 128], f32, tag="ptA")
            nc.tensor.transpose(ptu[:], u_t[:], idn[:])
            u_b = work.tile([BL, 128], f32, tag="u_b")
            nc.scalar.copy(u_b[:], ptu[:])

            rm1 = work.tile([BL, 128], f32, tag="rm1")
            rp2 = work.tile([BL, 128], f32, tag="rp2")
            rp1 = work.tile([BL, 128], f32, tag="rp1")
            nc.vector.tensor_copy(rm1[:, 0:127], u_b[:, 1:128])
            nc.vector.tensor_copy(rm1[:, 127:128], u_b[:, 0:1])
            nc.vector.tensor_copy(rp2[:, 2:128], u_b[:, 0:126])
            nc.vector.tensor_copy(rp2[:, 0:2], u_b[:, 126:128])
            nc.vector.tensor_copy(rp1[:, 1:128], u_b[:, 0:127])
            nc.vector.tensor_copy(rp1[:, 0:1], u_b[:, 127:128])
            sd = work.tile([BL, 128], f32, tag="sd")
            nc.vector.tensor_sub(sd[:], rm1[:], rp2[:])
            qd = work.tile([BL, 128], f32, tag="qd")
            nc.vector.tensor_mul(qd[:], sd[:], rp1[:])
            lor = work.tile([BL, 128], f32, tag="lor")
            # lor = (qd + F) - u_b
            nc.vector.scalar_tensor_tensor(
                lor[:], qd[:], float(F_LORENZ), u_b[:], ALU.add, ALU.subtract
            )
            ulor_b = work.tile([BL, 128], f32, tag="ulor_b")
            # ulor_b = (lor * DT) + u_b
            nc.vector.scalar_tensor_tensor(
                ulor_b[:], lor[:], float(DT), u_b[:], ALU.mult, ALU.add
            )

            # ---- softplus 1
            h1t = work.tile([128, 256], f32, tag="h1t")
            softplus(h1t, ps1, "sp1")

            # ---- mm2: h2T = W1 @ h1T, 16 matmuls, one bank group
            ps2 = ph2.tile([128, 256], f32, tag="ps2")
            for m in range(4):
                for k in range(4):
                    nc.tensor.matmul(
                        ps2[:, m * 64 : (m + 1) * 64],
                        lhsT=w1s[:, (k * 4 + m) * 128 : (k * 4 + m + 1) * 128],
                        rhs=h1t[:, k * 64 : (k + 1) * 64],
                        start=(m == 0 and k == 0),
                        stop=(m == 3 and k == 3),
                    )

            # ---- softplus 2
            h2t = work.tile([128, 256], f32, tag="h2t")
            softplus(h2t, ps2, "sp2")

            # ---- mm3 + tanh + einsum, per PSUM bank (8 d-tiles each)
            ebs = []
            for b in range(2):
                pso = po.tile([128, 512], f32, tag="pso")
                for t in range(8 * b, 8 * b + 8):
                    for k in range(4):
                        nc.tensor.matmul(
                            pso[:, (t - 8 * b) * 64 : (t - 8 * b + 1) * 64],
                            lhsT=w2s[:, (k * 16 + t) * 128 : (k * 16 + t + 1) * 128],
                            rhs=h2t[:, k * 64 : (k + 1) * 64],
                            start=(t == 8 * b and k == 0),
                            stop=(t == 8 * b + 7 and k == 3),
                        )
                # tanh(p) = 1 - 2*r, r = 1/(1+exp(2p)); affine part folded
                # into the einsum via the precomputed sum_d dX row.
                texp = work.tile([128, 512], f32, tag=f"texp{b}")
                nc.scalar.activation(texp[:], pso[:], AF.Exp, scale=2.0)
                tp1 = work.tile([128, 512], f32, tag=f"tp1{b}")
                nc.vector.tensor_scalar_add(tp1[:], texp[:], 1.0)
                r = work.tile([128, 512], f32, tag=f"r{b}")
                nc.vector.reciprocal(r[:], tp1[:])
                # P[p, b*8+d] = r[p, d*64+b] * dX[d,b]
                prod = work.tile([128, 512], f32, tag=f"prod{b}")
                row = dxr[:, b * 512 : (b + 1) * 512].rearrange(
                    "p (d b) -> p d b", d=8
                )
                nc.vector.tensor_tensor(
                    prod[:].rearrange("p (b d) -> p d b", d=8),
                    r[:].rearrange("p (d b) -> p d b", d=8),
                    row,
                    ALU.mult,
                )
                eb = work.tile([128, BL], f32, tag=f"eb{b}")
                nc.vector.tensor_reduce(
                    eb[:],
                    prod[:].rearrange("p (b d) -> p b d", d=8),
                    mybir.AxisListType.X,
                    ALU.add,
                )
                ebs.append(eb)

            # ---- transpose ulor back to feature-major (PE slot after mm3)
            ptl = pt.tile([128, BL], f32, tag="ptB")
            nc.tensor.transpose(ptl[:], ulor_b[:], idn[0:BL, 0:BL])
            ulort = work.tile([128, BL], f32, tag="ulort")
            nc.scalar.copy(ulort[:], ptl[:])

            # ---- state update: u_next = ulorT + sum_d dX - 2*(R0+R1)
            rsum = work.tile([128, BL], f32, tag="rsum")
            nc.vector.tensor_add(rsum[:], ebs[0][:], ebs[1][:])
            ulors = work.tile([128, BL], f32, tag="ulors")
            nc.vector.tensor_add(ulors[:], ulort[:], dxr[:, 1024:1088])
            u_next = state.tile([128, BL], f32, tag="u")
            nc.vector.scalar_tensor_tensor(
                u_next[:], rsum[:], -2.0, ulors[:], ALU.mult, ALU.add
            )
            u_t = u_next

        nc.sync.dma_start(out=out_d[:], in_=u_t[:])

    nc.compile()
    return nc


def _host_prep(u0, ts, coeff_a, coeff_b, coeff_c, coeff_d, W0, W1, W2, steps):
    """Exact-f32 dX precompute + per-core input maps."""
    n = np.arange(steps, dtype=np.float32)
    t0 = (ts[0] + n * DT).astype(np.float32)
    t1 = (t0 + DT).astype(np.float32)

    def interp(t):
        idx = np.clip(np.searchsorted(ts, t, side="right") - 1, 0, N - 2)
        frac = (t - ts[idx]).astype(np.float32)
        f = frac[None, :, None]
        a = coeff_a[:, idx]
        b = coeff_b[:, idx]
        c = coeff_c[:, idx]
        d = coeff_d[:, idx]
        return (a + f * (b + f * (c + f * d))).astype(np.float32)

    dX = (interp(t1) - interp(t0)).astype(np.float32)  # [B, steps, D]
    sdx = dX.sum(axis=2, dtype=np.float32)  # [B, steps]

    w0s = np.ascontiguousarray(W0.T)  # [128, 512]
    w1s = np.ascontiguousarray(
        W1.reshape(4, 128, 4, 128)
        .transpose(2, 0, 3, 1)
        .reshape(16, 128, 128)
        .transpose(1, 0, 2)
        .reshape(128, 2048)
    )
    w2r = W2.reshape(128, 16, 512).transpose(1, 0, 2).reshape(2048, 512)
    w2s = np.ascontiguousarray(
        w2r.reshape(16, 128, 4, 128)
        .transpose(2, 0, 3, 1)
        .reshape(64, 128, 128)
        .transpose(1, 0, 2)
        .reshape(128, 8192)
    )
    idn = np.eye(128, dtype=np.float32)

    in_maps = []
    for c in range(NCORES):
        rows = slice(c * BL, (c + 1) * BL)
        u0t = np.ascontiguousarray(u0[rows].T)
        dxt = np.empty((steps, 1088), np.float32)
        dxt[:, :1024] = dX[rows].transpose(1, 2, 0).reshape(steps, 1024)
        dxt[:, 1024:] = sdx[rows].T
        in_maps.append(
            dict(u0t=u0t, w0s=w0s, w1s=w1s, w2s=w2s, dxt=dxt, idn=idn)
        )
    return in_maps


def _kernel_np(u0, ts, coeff_a, coeff_b, coeff_c, coeff_d, W0, b0, W1, b1, W2, b2):
    """Plain numpy fallback (only used if biases are nonzero)."""
    n = np.arange(STEPS, dtype=np.float32)
    t0 = (ts[0] + n * DT).astype(np.float32)
    t1 = (t0 + DT).astype(np.float32)

    def interp(t):
        idx = np.clip(np.searchsorted(ts, t, side="right") - 1, 0, N - 2)
        f = (t - ts[idx]).astype(np.float32)[None, :, None]
        return (coeff_a[:, idx] + f * (coeff_b[:, idx] + f * (coeff_c[:, idx] + f * coeff_d[:, idx]))).astype(np.float32)

    dX = (interp(t1) - interp(t0)).astype(np.float32)
    W0T, W1T, W2T = W0.T.copy(), W1.T.copy(), W2.T.copy()

    def softplus(x):
        return np.logaddexp(np.float32(0.0), x).astype(np.float32)

    u = u0.copy()
    for s in range(STEPS):
        h = softplus(u @ W0T + b0)
        h = softplus(h @ W1T + b1)
        o = np.tanh(h @ W2T + b2).astype(np.float32)
        lor = ((np.roll(u, -1, 1) - np.roll(u, 2, 1)) * np.roll(u, 1, 1) - u + F_LORENZ).astype(np.float32)
        u = (u + lor * DT + np.einsum("bhd,bd->bh", o.reshape(B, H, D), dX[:, s])).astype(np.float32)
    return u.astype(np.float32)


def kernel(u0, ts, coeff_a, coeff_b, coeff_c, coeff_d, W0, b0, W1, b1, W2, b2):
    u0 = np.asarray(u0, np.float32)
    ts = np.asarray(ts, np.float32)
    coeff_a = np.asarray(coeff_a, np.float32)
    coeff_b = np.asarray(coeff_b, np.float32)
    coeff_c = np.asarray(coeff_c, np.float32)
    coeff_d = np.asarray(coeff_d, np.float32)
    W0 = np.asarray(W0, np.float32)
    W1 = np.asarray(W1, np.float32)
    W2 = np.asarray(W2, np.float32)
    b0 = np.asarray(b0, np.float32)
    b1 = np.asarray(b1, np.float32)
    b2 = np.asarray(b2, np.float32)

    if np.any(b0) or np.any(b1) or np.any(b2):
        return _kernel_np(u0, ts, coeff_a, coeff_b, coeff_c, coeff_d, W0, b0, W1, b1, W2, b2)

    _, _, _, _, run_bass_kernel_spmd = _import_concourse()
    _patch_neff_cache()

    if "nc" not in _STATE:
        t0c = time.perf_counter()
        _STATE["nc"] = _build(STEPS)
        _STATE["build_s"] = time.perf_counter() - t0c

    in_maps = _host_prep(u0, ts, coeff_a, coeff_b, coeff_c, coeff_d, W0, W1, W2, STEPS)

    t0r = time.perf_counter()
    res = run_bass_kernel_spmd(
        _STATE["nc"], in_maps, core_ids=list(range(NCORES)),
        trace=bool(int(os.environ.get("NEURALCDE_TRACE", "0"))),
    )
    _STATE["run_s"] = time.perf_counter() - t0r
    _STATE["exec_time_ns"] = res.exec_time_ns

    out = np.empty((B, H), np.float32)
    for c in range(NCORES):
        out[c * BL : (c + 1) * BL] = res.results[c]["out"].T
    return out
